# revision 1
# baseline (speedup 1.0000x reference)
# Multi-head attention (K/Q swapped variant) on 8 Trainium2 NeuronCores.
#
# Sharding: core = b*2 + half, b = batch (4), half = which 1024-row slice of
# the output sequence this core produces. Each core computes all 16 heads for
# its (batch, s-slice) and the final out-projection rows, so per-core outputs
# concatenate exactly into the full result (no cross-core reduction).
#
# Math (per batch b, head h), matching the reference exactly:
#   q[t] = (x[t] @ Wq.T + bq)/8 ; k[s] = x[s] @ Wk.T + bk
#   scoresT[t,s] = q[t] . k[s]        (= reference scores[s,t])
#   P[t,s] = exp(scoresT[t,s]) * mask[b,0,s,t]      (no max-subtraction:
#       scores are O(1) by construction; masked entries exact 0)
#   XP[d',s] = sum_t x_ext[t,d'] P[t,s]   (x_ext has a ones column, so
#       XP[64,s] = sum_t P[t,s] = softmax denominator)
#   occ = XP[0:64]/denom ; y = sum_h occ_h.T @ Weff_h + bo'
#       where Weff_h = Wv.T @ Wo[:, h*64:(h+1)*64].T (Wv folded into Wo on
#       the host; bv's contribution rides bo' since sum_t attn = 1).
#
# Perf notes (measured on this hw):
#  - matmul cost ~= 0.52 ns per output column when the contraction uses 128
#    partitions; ~0.91 when it uses 64. So q/k are zero-padded to K=128.
#  - q/k live in fp8e4 (sqrt(1/8) scale folded into each side): halves the
#    PE's score-operand SBUF traffic; adds ~0.8% rel err (gate is 2e-2).
#  - q|k projections are packed into one [65,128] stationary (q rows 0:63,
#    k rows 64:127 of each PSUM tile).
#  - exp runs on ACT in [128,1024] tiles; mask multiply runs on DVE.
#  - XP is computed transposed (lhsT = P chunk, rhs = x_ext, N=65) into
#    region-opened PSUM accumulators, normalized by a per-partition
#    reciprocal, then PE-transposed back via an identity matmul.
import numpy as np

import concourse.bass as bass
import concourse.bacc as bacc
import concourse.mybir as mybir
import concourse.tile as tile
from concourse.bass_utils import run_bass_kernel_spmd

B, S, MD, NH, D = 4, 2048, 1024, 16, 64
SH = S // 2          # per-core output rows
TC = S // 128        # 16 t-chunks
F32 = mybir.dt.float32
F16 = mybir.dt.float16
F8 = mybir.dt.float8e4

_BUILD_CACHE = {}


def _build(loop_n=1):
    if loop_n in _BUILD_CACHE:
        return _BUILD_CACHE[loop_n]
    nc = bacc.Bacc("TRN2", target_bir_lowering=False, debug=False)

    xTq_d = nc.dram_tensor("xTq", [NH, D + 1, S], F16, kind="ExternalInput")
    xe_d = nc.dram_tensor("xe", [NH, 128, TC, D + 1], F16, kind="ExternalInput")
    mT_d = nc.dram_tensor("maskT", [128, TC, SH], F16, kind="ExternalInput")
    weff_d = nc.dram_tensor("weff", [MD, MD], F16, kind="ExternalInput")
    bo2_d = nc.dram_tensor("bo2", [1, MD], F32, kind="ExternalInput")
    wqk_d = nc.dram_tensor("wqk", [D + 1, 128], F16, kind="ExternalInput")
    ident_d = nc.dram_tensor("ident", [128, 128], F16, kind="ExternalInput")
    y_d = nc.dram_tensor("y", [SH, MD], F32, kind="ExternalOutput")

    with tile.TileContext(nc) as tc:
        with tc.tile_pool(name="consts", bufs=1) as consts:
            weff_sb = consts.tile([128, 8, MD], F16, tag="weff")
            for ec in range(8):
                nc.gpsimd.dma_start(
                    out=weff_sb[:, ec, :],
                    in_=weff_d.ap().rearrange("(ec p) m -> p ec m", p=128)[:, ec, :],
                )
            mT_sb = consts.tile([128, TC, SH], F16, tag="mT")
            for c in range(TC):
                nc.gpsimd.dma_start(out=mT_sb[:, c, :], in_=mT_d.ap()[:, c, :])
            wqk_sb = consts.tile([D + 1, 128], F16, tag="wqk")
            nc.sync.dma_start(out=wqk_sb[:], in_=wqk_d.ap())
            bo_bc = consts.tile([128, MD], F32, tag="bo")
            bo_ap = bo2_d.ap()[0:1, :]
            nc.gpsimd.dma_start(
                out=bo_bc[:],
                in_=bass.AP(
                    tensor=bo_ap.tensor,
                    offset=bo_ap.offset,
                    ap=[[0, 128]] + bo_ap.ap[1:],
                ),
            )
            occ = [consts.tile([128, SH], F16, tag=f"occ{c}", name=f"occ{c}") for c in range(8)]
            ident_sb = consts.tile([128, 128], F16, tag="ident")
            nc.sync.dma_start(out=ident_sb[:], in_=ident_d.ap())

            def body(_iv=None):
                with (
                    tc.tile_pool(name="xin", bufs=2) as xin,
                    tc.tile_pool(name="pp", bufs=5) as pp,
                    tc.tile_pool(name="rct", bufs=2) as rctp,
                    tc.tile_pool(name="xpn", bufs=2) as xpnp,
                    tc.tile_pool(name="pq", bufs=1, space="PSUM") as pqp,
                    tc.tile_pool(name="scp", bufs=2, space="PSUM") as scp,
                    tc.tile_pool(name="xpp", bufs=1, space="PSUM") as xpp,
                    tc.tile_pool(name="tpp", bufs=1, space="PSUM") as tpp,
                ):
                    # q16/k16 double buffers: K-padding rows zeroed once,
                    # rotated manually so the zero rows persist
                    q16_bufs = []
                    k16_bufs = []
                    for z in range(2):
                        zq = consts.tile([128, S], F8, tag=f"q16{z}", name=f"q16{z}")
                        nc.vector.memset(zq[64:128, :], 0.0)
                        q16_bufs.append(zq)
                        zk = consts.tile([128, SH], F8, tag=f"k16{z}", name=f"k16{z}")
                        nc.vector.memset(zk[64:128, :], 0.0)
                        k16_bufs.append(zk)

                    def emit_proj_dma(h):
                        xTq_sb = xin.tile([D + 1, S], F16, tag="xq", name="xTq_sb")
                        for j in range(2):
                            nc.sync.dma_start(
                                out=xTq_sb[:, j * SH : (j + 1) * SH],
                                in_=xTq_d.ap()[h][:, j * SH : (j + 1) * SH],
                            )
                        return xTq_sb

                    def emit_proj_mm(h, xTq_sb, jj):
                        q16 = q16_bufs[h % 2]
                        k16 = k16_bufs[h % 2]
                        pq = pqp.tile([128, 512], F32, tag="pq", name="pq")
                        nc.tensor.matmul(
                            pq[:],
                            wqk_sb[:],
                            xTq_sb[:, jj * 512 : (jj + 1) * 512],
                            start=True,
                            stop=True,
                        )
                        nc.vector.tensor_copy(
                            q16[0:64, jj * 512 : (jj + 1) * 512], pq[0:64, :]
                        )
                        if jj < 2:
                            nc.vector.tensor_copy(
                                k16[0:64, jj * 512 : (jj + 1) * 512],
                                pq[64:128, :],
                            )
                        return q16, k16

                    xe_all = consts.tile(
                        [128, NH, TC, D + 1], F16, tag="xeall", name="xe_all"
                    )
                    for hh in range(NH):
                        nc.sync.dma_start(
                            out=xe_all[:, hh, :, :], in_=xe_d.ap()[hh]
                        )
                    xTq0 = emit_proj_dma(0)
                    for jj in range(4):
                        emit_proj_mm(0, xTq0, jj)
                    qk_tiles = {0: (q16_bufs[0], k16_bufs[0])}
                    prev_xpn = None
                    for h in range(NH):
                        xe_sb = xe_all[:, h, :, :]
                        q16, k16 = qk_tiles.pop(h)

                        # XP^T accumulator: [s-in-chunk, si, e] over 16
                        # t-chunks; col 64 = softmax denominator. Region-open
                        # trick: one N=1 start=True matmul per 2KB zero
                        # region, then all real accumulators use start=False.
                        acc = xpp.tile([128, 8, 128], F32, tag="xp")

                        def emit_opens():
                            for reg in range(2):
                                nc.tensor.matmul(
                                    acc[:, reg * 4, 65:66],
                                    wqk_sb[:],
                                    wqk_sb[:, 0:1],
                                    start=True,
                                    stop=False,
                                    skip_group_check=True,
                                )

                        def emit_xpt(c, ptc, last):
                            for si in range(8):
                                nc.tensor.matmul(
                                    acc[:, si, 0:65],
                                    ptc[:, si * 128 : (si + 1) * 128],
                                    xe_sb[:, c, :],
                                    start=False,
                                    stop=(last and si == 7),
                                    skip_group_check=True,
                                )

                        def emit_head_end(hh):
                            # normalize by 1/denom, transpose to occ layout
                            rc_t = rctp.tile([128, 8], F32, tag="rct")
                            nc.vector.reciprocal_approx_fast(
                                out=rc_t[:], in_=acc[:, :, 64]
                            )
                            xpn_t = xpnp.tile([128, 8, 64], F16, tag="xpn")
                            rcb = rc_t[:]
                            rc_bc = bass.AP(
                                tensor=rcb.tensor,
                                offset=rcb.offset,
                                ap=[rcb.ap[0], [1, 8], [0, 64]],
                            )
                            nc.vector.tensor_mul(
                                xpn_t[:], acc[:, :, 0:64], rc_bc
                            )
                            return xpn_t

                        def emit_transposes(hh, xpn_t):
                            tp = tpp.tile([64, 8, 128], F16, tag="tp")
                            for si in range(8):
                                nc.tensor.matmul(
                                    tp[:, si, :],
                                    xpn_t[:, si, :],
                                    ident_sb[:],
                                    is_transpose=True,
                                    skip_group_check=True,
                                )
                            ci, half = hh // 2, hh % 2
                            nc.vector.tensor_copy(
                                occ[ci][half * 64 : (half + 1) * 64, :],
                                tp[:].rearrange("p a b -> p (a b)"),
                            )

                        pt_tiles = {}
                        for c in range(TC):
                            sc = scp.tile([128, SH], F32, tag="sc", name="sc")
                            for jj in (0, 512):
                                nc.tensor.matmul(
                                    sc[:, jj : jj + 512],
                                    q16[:, c * 128 : (c + 1) * 128],
                                    k16[:, jj : jj + 512],
                                    start=True,
                                    stop=True,
                                )
                            if c == 1:
                                if prev_xpn is not None:
                                    emit_transposes(h - 1, prev_xpn)
                                    prev_xpn = None
                                emit_opens()
                            pt = pp.tile([128, SH], F16, tag="pt")
                            nc.scalar.activation(
                                pt[:], sc[:], mybir.ActivationFunctionType.Exp
                            )
                            nc.vector.tensor_mul(pt[:], pt[:], mT_sb[:, c, :])
                            pt_tiles[c] = pt
                            if c >= 2:
                                emit_xpt(c - 2, pt_tiles.pop(c - 2), False)
                            if c == 4 and h + 1 < NH:
                                qk_tiles[h + 1] = (
                                    q16_bufs[(h + 1) % 2],
                                    k16_bufs[(h + 1) % 2],
                                )
                                xTq_next = emit_proj_dma(h + 1)
                            if 5 <= c <= 8 and h + 1 < NH:
                                emit_proj_mm(h + 1, xTq_next, c - 5)
                        emit_xpt(TC - 2, pt_tiles.pop(TC - 2), False)
                        emit_xpt(TC - 1, pt_tiles.pop(TC - 1), True)
                        prev_xpn = emit_head_end(h)
                        if h == NH - 1:
                            emit_transposes(h, prev_xpn)

                with (
                    tc.tile_pool(name="fin", bufs=2, space="PSUM") as fin,
                    tc.tile_pool(name="ysb", bufs=2) as ysb,
                ):
                    for si in range(8):
                        yp = fin.tile([128, MD], F32, tag="fin")
                        for jj in (0, 512):
                            for c_idx in range(8):
                                nc.tensor.matmul(
                                    yp[:, jj : jj + 512],
                                    occ[c_idx][:, si * 128 : (si + 1) * 128],
                                    weff_sb[:, c_idx, jj : jj + 512],
                                    start=(c_idx == 0),
                                    stop=(c_idx == 7),
                                )
                        y_sb = ysb.tile([128, MD], F32, tag="ysb")
                        nc.vector.tensor_add(y_sb[:], yp[:], bo_bc[:])
                        nc.sync.dma_start(
                            out=y_d.ap()[si * 128 : (si + 1) * 128, :], in_=y_sb[:]
                        )

            if loop_n > 1:
                with tc.For_i(0, loop_n, 1):
                    body()
            else:
                body()

    nc.compile()
    _BUILD_CACHE[loop_n] = nc
    return nc


def _prep(input, mask, Wk, bk, Wq, bq, Wv, bv, Wo, bo):
    x = np.ascontiguousarray(np.asarray(input, np.float32))
    mask = np.asarray(mask)
    f32 = np.float32

    wq_ext = np.concatenate(
        [np.asarray(Wq, f32).T, np.asarray(bq, f32)[None, :]], axis=0
    ) * f32(0.35355339)
    wk_ext = np.concatenate(
        [np.asarray(Wk, f32).T, np.asarray(bk, f32)[None, :]], axis=0
    ) * f32(0.35355339)
    wqk = np.concatenate([wq_ext, wk_ext], axis=1)  # [65, 128]

    WvT = np.asarray(Wv, f32).T                      # [64 d, 64 d']
    Wo_f = np.asarray(Wo, f32)                       # [MD, MD]
    Wo_blocks = Wo_f.reshape(MD, NH, D)              # [m, h, d']
    weff = np.einsum("dD,mhD->hdm", WvT, Wo_blocks).reshape(MD, MD)
    bo2 = (np.asarray(bo, f32) + np.tile(np.asarray(bv, f32), NH) @ Wo_f.T).reshape(
        1, MD
    )

    shared = {
        "ident": np.eye(128, dtype=np.float16),
        "wqk": np.ascontiguousarray(wqk).astype(np.float16),
        "weff": np.ascontiguousarray(weff).astype(np.float16),
        "bo2": np.ascontiguousarray(bo2).astype(np.float32),
    }

    per_batch = []
    for b in range(B):
        xb = x[b]  # [S, MD]
        xTq = np.empty((NH, D + 1, S), np.float16)
        xTq[:, :D, :] = xb.T.reshape(NH, D, S)
        xTq[:, D, :] = 1.0
        xe = np.empty((NH, 128, TC, D + 1), np.float16)
        # [c,p,h,d] -> [h,p,c,d]
        xe[:, :, :, :D] = xb.reshape(TC, 128, NH, D).transpose(2, 1, 0, 3)
        xe[:, :, :, D] = 1.0
        per_batch.append((xTq, xe, np.asarray(mask[b, 0])))

    in_maps = []
    for core in range(8):
        b, half = core // 2, core % 2
        s0 = half * SH
        xTq, xe, mb = per_batch[b]
        # per-core t-permutation: local s-half chunks first
        if half == 0:
            xTq_p, xe_p = xTq, xe
        else:
            xTq_p = np.concatenate([xTq[:, :, SH:], xTq[:, :, :SH]], axis=2)
            xe_p = np.concatenate([xe[:, :, 8:, :], xe[:, :, :8, :]], axis=2)
        # maskT[p, c, sl] = mask[s0+sl, t(c)*128+p] with permuted t-chunk order
        mT = np.ascontiguousarray(
            mb[s0 : s0 + SH, :].reshape(SH, TC, 128).transpose(2, 1, 0)
        ).astype(np.float16)
        if half == 1:
            mT = np.ascontiguousarray(
                np.concatenate([mT[:, 8:, :], mT[:, :8, :]], axis=1)
            )
        in_maps.append(
            dict(
                shared,
                xTq=np.ascontiguousarray(xTq_p),
                xe=np.ascontiguousarray(xe_p),
                maskT=mT,
            )
        )
    return in_maps


def _assemble(results):
    y = np.empty((B, S, MD), np.float32)
    for core in range(8):
        b, half = core // 2, core % 2
        y[b, half * SH : (half + 1) * SH, :] = results[core]["y"]
    return y


def kernel(input, mask, Wk, bk, Wq, bq, Wv, bv, Wo, bo):
    in_maps = _prep(input, mask, Wk, bk, Wq, bq, Wv, bv, Wo, bo)
    nc = _build(1)
    res = run_bass_kernel_spmd(nc, in_maps, list(range(8)))
    return _assemble(res.results)


def timed_run(inputs, loop_n):
    """Run with the body repeated loop_n times on-device; returns wall seconds."""
    import time

    in_maps = _prep(**inputs)
    nc = _build(loop_n)
    t0 = time.perf_counter()
    res = run_bass_kernel_spmd(nc, in_maps, list(range(8)))
    t1 = time.perf_counter()
    return t1 - t0, _assemble(res.results)



# revision 4
# speedup vs baseline: 1.0001x; 1.0001x over previous
# Multi-head attention (K/Q swapped variant) on 8 Trainium2 NeuronCores.
#
# Sharding: core = b*2 + half, b = batch (4), half = which 1024-row slice of
# the output sequence this core produces. Each core computes all 16 heads for
# its (batch, s-slice) and the final out-projection rows, so per-core outputs
# concatenate exactly into the full result (no cross-core reduction).
#
# Math (per batch b, head h), matching the reference exactly:
#   q[t] = (x[t] @ Wq.T + bq)/8 ; k[s] = x[s] @ Wk.T + bk
#   scoresT[t,s] = q[t] . k[s]        (= reference scores[s,t])
#   P[t,s] = exp(scoresT[t,s]) * mask[b,0,s,t]      (no max-subtraction:
#       scores are O(1) by construction; masked entries exact 0)
#   XP[d',s] = sum_t x_ext[t,d'] P[t,s]   (x_ext has a ones column, so
#       XP[64,s] = sum_t P[t,s] = softmax denominator)
#   occ = XP[0:64]/denom ; y = sum_h occ_h.T @ Weff_h + bo'
#       where Weff_h = Wv.T @ Wo[:, h*64:(h+1)*64].T (Wv folded into Wo on
#       the host; bv's contribution rides bo' since sum_t attn = 1).
#
# Perf notes (measured on this hw):
#  - matmul cost ~= 0.52 ns per output column when the contraction uses 128
#    partitions; ~0.91 when it uses 64. So q/k are zero-padded to K=128.
#  - q/k live in fp8e4 (sqrt(1/8) scale folded into each side): halves the
#    PE's score-operand SBUF traffic; adds ~0.8% rel err (gate is 2e-2).
#  - q|k projections are packed into one [65,128] stationary (q rows 0:63,
#    k rows 64:127 of each PSUM tile).
#  - exp runs on ACT in [128,1024] tiles; mask multiply runs on DVE.
#  - XP is computed transposed (lhsT = P chunk, rhs = x_ext, N=65) into
#    region-opened PSUM accumulators, normalized by a per-partition
#    reciprocal, then PE-transposed back via an identity matmul.
import numpy as np

import concourse.bass as bass
import concourse.bacc as bacc
import concourse.mybir as mybir
import concourse.tile as tile
from concourse.bass_utils import run_bass_kernel_spmd

B, S, MD, NH, D = 4, 2048, 1024, 16, 64
SH = S // 2          # per-core output rows
TC = S // 128        # 16 t-chunks
F32 = mybir.dt.float32
F16 = mybir.dt.float16
F8 = mybir.dt.float8e4

_BUILD_CACHE = {}


def _build(loop_n=1):
    if loop_n in _BUILD_CACHE:
        return _BUILD_CACHE[loop_n]
    nc = bacc.Bacc("TRN2", target_bir_lowering=False, debug=False)

    xTq_d = nc.dram_tensor("xTq", [NH, D + 1, S], F16, kind="ExternalInput")
    xe_d = nc.dram_tensor("xe", [NH, 128, TC, D + 1], F16, kind="ExternalInput")
    mT_d = nc.dram_tensor("maskT", [128, TC, SH], F16, kind="ExternalInput")
    weff_d = nc.dram_tensor("weff", [MD, MD], F16, kind="ExternalInput")
    bo2_d = nc.dram_tensor("bo2", [1, MD], F32, kind="ExternalInput")
    wqk_d = nc.dram_tensor("wqk", [D + 1, 128], F16, kind="ExternalInput")
    ident_d = nc.dram_tensor("ident", [128, 128], F16, kind="ExternalInput")
    y_d = nc.dram_tensor("y", [SH, MD], F32, kind="ExternalOutput")

    with tile.TileContext(nc) as tc:
        with tc.tile_pool(name="consts", bufs=1) as consts:
            weff_sb = consts.tile([128, 8, MD], F16, tag="weff")
            for ec in range(8):
                nc.gpsimd.dma_start(
                    out=weff_sb[:, ec, :],
                    in_=weff_d.ap().rearrange("(ec p) m -> p ec m", p=128)[:, ec, :],
                )
            mT_sb = consts.tile([128, TC, SH], F16, tag="mT")
            for c in range(TC):
                nc.gpsimd.dma_start(out=mT_sb[:, c, :], in_=mT_d.ap()[:, c, :])
            wqk_sb = consts.tile([D + 1, 128], F16, tag="wqk")
            nc.sync.dma_start(out=wqk_sb[:], in_=wqk_d.ap())
            bo_bc = consts.tile([128, MD], F32, tag="bo")
            bo_ap = bo2_d.ap()[0:1, :]
            nc.gpsimd.dma_start(
                out=bo_bc[:],
                in_=bass.AP(
                    tensor=bo_ap.tensor,
                    offset=bo_ap.offset,
                    ap=[[0, 128]] + bo_ap.ap[1:],
                ),
            )
            occ = [consts.tile([128, SH], F16, tag=f"occ{c}", name=f"occ{c}") for c in range(8)]
            ident_sb = consts.tile([128, 128], F16, tag="ident")
            nc.sync.dma_start(out=ident_sb[:], in_=ident_d.ap())

            def body(_iv=None):
                with (
                    tc.tile_pool(name="xin", bufs=2) as xin,
                    tc.tile_pool(name="pp", bufs=5) as pp,
                    tc.tile_pool(name="rct", bufs=2) as rctp,
                    tc.tile_pool(name="xpn", bufs=2) as xpnp,
                    tc.tile_pool(name="pq", bufs=1, space="PSUM") as pqp,
                    tc.tile_pool(name="scp", bufs=2, space="PSUM") as scp,
                    tc.tile_pool(name="xpp", bufs=1, space="PSUM") as xpp,
                    tc.tile_pool(name="tpp", bufs=1, space="PSUM") as tpp,
                ):
                    # q16/k16 double buffers: K-padding rows zeroed once,
                    # rotated manually so the zero rows persist
                    # DoubleRow fp8 layout: [128, 2, N] with data in subtile 0
                    # rows 0:64; subtile 1 and rows 64:128 stay zero.
                    q16_bufs = []
                    k16_bufs = []
                    for z in range(2):
                        zq = consts.tile([128, 2, S], F8, tag=f"q16{z}", name=f"q16{z}")
                        nc.vector.memset(zq[:], 0.0)
                        q16_bufs.append(zq)
                        zk = consts.tile([128, 2, SH], F8, tag=f"k16{z}", name=f"k16{z}")
                        nc.vector.memset(zk[:], 0.0)
                        k16_bufs.append(zk)

                    def emit_proj_dma(h):
                        xTq_sb = xin.tile([D + 1, S], F16, tag="xq", name="xTq_sb")
                        for j in range(2):
                            nc.sync.dma_start(
                                out=xTq_sb[:, j * SH : (j + 1) * SH],
                                in_=xTq_d.ap()[h][:, j * SH : (j + 1) * SH],
                            )
                        return xTq_sb

                    def emit_proj_mm(h, xTq_sb, jj):
                        q16 = q16_bufs[h % 2]
                        k16 = k16_bufs[h % 2]
                        pq = pqp.tile([128, 512], F32, tag="pq", name="pq")
                        nc.tensor.matmul(
                            pq[:],
                            wqk_sb[:],
                            xTq_sb[:, jj * 512 : (jj + 1) * 512],
                            start=True,
                            stop=True,
                        )
                        nc.vector.tensor_copy(
                            q16[0:64, 0, jj * 512 : (jj + 1) * 512], pq[0:64, :]
                        )
                        if jj < 2:
                            nc.vector.tensor_copy(
                                k16[0:64, 0, jj * 512 : (jj + 1) * 512],
                                pq[64:128, :],
                            )
                        return q16, k16

                    xe_all = consts.tile(
                        [128, NH, TC, D + 1], F16, tag="xeall", name="xe_all"
                    )
                    for hh in range(NH):
                        nc.sync.dma_start(
                            out=xe_all[:, hh, :, :], in_=xe_d.ap()[hh]
                        )
                    xTq0 = emit_proj_dma(0)
                    for jj in range(4):
                        emit_proj_mm(0, xTq0, jj)
                    qk_tiles = {0: (q16_bufs[0], k16_bufs[0])}
                    prev_xpn = None
                    for h in range(NH):
                        xe_sb = xe_all[:, h, :, :]
                        q16, k16 = qk_tiles.pop(h)

                        # XP^T accumulator: [s-in-chunk, si, e] over 16
                        # t-chunks; col 64 = softmax denominator. Region-open
                        # trick: one N=1 start=True matmul per 2KB zero
                        # region, then all real accumulators use start=False.
                        acc = xpp.tile([128, 8, 128], F32, tag="xp")

                        def emit_opens():
                            for reg in range(2):
                                nc.tensor.matmul(
                                    acc[:, reg * 4, 65:66],
                                    wqk_sb[:],
                                    wqk_sb[:, 0:1],
                                    start=True,
                                    stop=False,
                                    skip_group_check=True,
                                )

                        def emit_xpt(c, ptc, last):
                            for si in range(8):
                                nc.tensor.matmul(
                                    acc[:, si, 0:65],
                                    ptc[:, si * 128 : (si + 1) * 128],
                                    xe_sb[:, c, :],
                                    start=False,
                                    stop=(last and si == 7),
                                    skip_group_check=True,
                                )

                        def emit_head_end(hh):
                            # normalize by 1/denom, transpose to occ layout
                            rc_t = rctp.tile([128, 8], F32, tag="rct")
                            nc.vector.reciprocal_approx_fast(
                                out=rc_t[:], in_=acc[:, :, 64]
                            )
                            xpn_t = xpnp.tile([128, 8, 64], F16, tag="xpn")
                            rcb = rc_t[:]
                            rc_bc = bass.AP(
                                tensor=rcb.tensor,
                                offset=rcb.offset,
                                ap=[rcb.ap[0], [1, 8], [0, 64]],
                            )
                            nc.vector.tensor_mul(
                                xpn_t[:], acc[:, :, 0:64], rc_bc
                            )
                            return xpn_t

                        def emit_transposes(hh, xpn_t):
                            tp = tpp.tile([64, 8, 128], F16, tag="tp")
                            for si in range(8):
                                nc.tensor.matmul(
                                    tp[:, si, :],
                                    xpn_t[:, si, :],
                                    ident_sb[:],
                                    is_transpose=True,
                                    skip_group_check=True,
                                )
                            ci, half = hh // 2, hh % 2
                            nc.vector.tensor_copy(
                                occ[ci][half * 64 : (half + 1) * 64, :],
                                tp[:].rearrange("p a b -> p (a b)"),
                            )

                        pt_tiles = {}
                        for c in range(TC):
                            sc = scp.tile([128, SH], F32, tag="sc", name="sc")
                            for jj in (0, 512):
                                nc.tensor.matmul(
                                    sc[:, jj : jj + 512],
                                    q16[:, :, c * 128 : (c + 1) * 128],
                                    k16[:, :, jj : jj + 512],
                                    start=True,
                                    stop=True,
                                    perf_mode=mybir.MatmulPerfMode.DoubleRow,
                                )
                            if c == 1:
                                if prev_xpn is not None:
                                    emit_transposes(h - 1, prev_xpn)
                                    prev_xpn = None
                                emit_opens()
                            pt = pp.tile([128, SH], F16, tag="pt")
                            nc.scalar.activation(
                                pt[:], sc[:], mybir.ActivationFunctionType.Exp
                            )
                            nc.vector.tensor_mul(pt[:], pt[:], mT_sb[:, c, :])
                            pt_tiles[c] = pt
                            if c >= 2:
                                emit_xpt(c - 2, pt_tiles.pop(c - 2), False)
                            if c == 4 and h + 1 < NH:
                                qk_tiles[h + 1] = (
                                    q16_bufs[(h + 1) % 2],
                                    k16_bufs[(h + 1) % 2],
                                )
                                xTq_next = emit_proj_dma(h + 1)
                            if 5 <= c <= 8 and h + 1 < NH:
                                emit_proj_mm(h + 1, xTq_next, c - 5)
                        emit_xpt(TC - 2, pt_tiles.pop(TC - 2), False)
                        emit_xpt(TC - 1, pt_tiles.pop(TC - 1), True)
                        prev_xpn = emit_head_end(h)
                        if h == NH - 1:
                            emit_transposes(h, prev_xpn)

                with (
                    tc.tile_pool(name="fin", bufs=2, space="PSUM") as fin,
                    tc.tile_pool(name="ysb", bufs=2) as ysb,
                ):
                    for si in range(8):
                        yp = fin.tile([128, MD], F32, tag="fin")
                        for jj in (0, 512):
                            for c_idx in range(8):
                                nc.tensor.matmul(
                                    yp[:, jj : jj + 512],
                                    occ[c_idx][:, si * 128 : (si + 1) * 128],
                                    weff_sb[:, c_idx, jj : jj + 512],
                                    start=(c_idx == 0),
                                    stop=(c_idx == 7),
                                )
                        y_sb = ysb.tile([128, MD], F32, tag="ysb")
                        nc.vector.tensor_add(y_sb[:], yp[:], bo_bc[:])
                        nc.sync.dma_start(
                            out=y_d.ap()[si * 128 : (si + 1) * 128, :], in_=y_sb[:]
                        )

            if loop_n > 1:
                with tc.For_i(0, loop_n, 1):
                    body()
            else:
                body()

    nc.compile()
    _BUILD_CACHE[loop_n] = nc
    return nc


def _prep(input, mask, Wk, bk, Wq, bq, Wv, bv, Wo, bo):
    x = np.ascontiguousarray(np.asarray(input, np.float32))
    mask = np.asarray(mask)
    f32 = np.float32

    wq_ext = np.concatenate(
        [np.asarray(Wq, f32).T, np.asarray(bq, f32)[None, :]], axis=0
    ) * f32(0.35355339)
    wk_ext = np.concatenate(
        [np.asarray(Wk, f32).T, np.asarray(bk, f32)[None, :]], axis=0
    ) * f32(0.35355339)
    wqk = np.concatenate([wq_ext, wk_ext], axis=1)  # [65, 128]

    WvT = np.asarray(Wv, f32).T                      # [64 d, 64 d']
    Wo_f = np.asarray(Wo, f32)                       # [MD, MD]
    Wo_blocks = Wo_f.reshape(MD, NH, D)              # [m, h, d']
    weff = np.einsum("dD,mhD->hdm", WvT, Wo_blocks).reshape(MD, MD)
    bo2 = (np.asarray(bo, f32) + np.tile(np.asarray(bv, f32), NH) @ Wo_f.T).reshape(
        1, MD
    )

    shared = {
        "ident": np.eye(128, dtype=np.float16),
        "wqk": np.ascontiguousarray(wqk).astype(np.float16),
        "weff": np.ascontiguousarray(weff).astype(np.float16),
        "bo2": np.ascontiguousarray(bo2).astype(np.float32),
    }

    per_batch = []
    for b in range(B):
        xb = x[b]  # [S, MD]
        xTq = np.empty((NH, D + 1, S), np.float16)
        xTq[:, :D, :] = xb.T.reshape(NH, D, S)
        xTq[:, D, :] = 1.0
        xe = np.empty((NH, 128, TC, D + 1), np.float16)
        # [c,p,h,d] -> [h,p,c,d]
        xe[:, :, :, :D] = xb.reshape(TC, 128, NH, D).transpose(2, 1, 0, 3)
        xe[:, :, :, D] = 1.0
        per_batch.append((xTq, xe, np.asarray(mask[b, 0])))

    in_maps = []
    for core in range(8):
        b, half = core // 2, core % 2
        s0 = half * SH
        xTq, xe, mb = per_batch[b]
        # per-core t-permutation: local s-half chunks first
        if half == 0:
            xTq_p, xe_p = xTq, xe
        else:
            xTq_p = np.concatenate([xTq[:, :, SH:], xTq[:, :, :SH]], axis=2)
            xe_p = np.concatenate([xe[:, :, 8:, :], xe[:, :, :8, :]], axis=2)
        # maskT[p, c, sl] = mask[s0+sl, t(c)*128+p] with permuted t-chunk order
        mT = np.ascontiguousarray(
            mb[s0 : s0 + SH, :].reshape(SH, TC, 128).transpose(2, 1, 0)
        ).astype(np.float16)
        if half == 1:
            mT = np.ascontiguousarray(
                np.concatenate([mT[:, 8:, :], mT[:, :8, :]], axis=1)
            )
        in_maps.append(
            dict(
                shared,
                xTq=np.ascontiguousarray(xTq_p),
                xe=np.ascontiguousarray(xe_p),
                maskT=mT,
            )
        )
    return in_maps


def _assemble(results):
    y = np.empty((B, S, MD), np.float32)
    for core in range(8):
        b, half = core // 2, core % 2
        y[b, half * SH : (half + 1) * SH, :] = results[core]["y"]
    return y


def kernel(input, mask, Wk, bk, Wq, bq, Wv, bv, Wo, bo):
    in_maps = _prep(input, mask, Wk, bk, Wq, bq, Wv, bv, Wo, bo)
    nc = _build(1)
    res = run_bass_kernel_spmd(nc, in_maps, list(range(8)))
    return _assemble(res.results)


def timed_run(inputs, loop_n):
    """Run with the body repeated loop_n times on-device; returns wall seconds."""
    import time

    in_maps = _prep(**inputs)
    nc = _build(loop_n)
    t0 = time.perf_counter()
    res = run_bass_kernel_spmd(nc, in_maps, list(range(8)))
    t1 = time.perf_counter()
    return t1 - t0, _assemble(res.results)

